# revision 9
# baseline (speedup 1.0000x reference)
"""MultiHeadAttention Trainium2 kernel (8 NeuronCores).

Problem: B=2, S=2048, E=1024, H=16, HD=64.
  qg = q @ Wq + bq ; qh[h] = qg @ Whq[h] + bhq[h]   (same for k, v)
  scores = qh @ kh^T / sqrt(HD), masked (-inf where mask), softmax
  out = concat_h(softmax @ vh) @ Wo + bo

Sharding: core c = 4*b + g handles batch b, heads 4g..4g+3 (data parallel on
B, tensor parallel on H). The global+per-head projections are folded on the
host into per-head fused weights Fq[h] = Wq @ Whq[h] (etc.), so each core
runs one [E, HD] projection per head. The output projection is row-sharded:
each core computes Wo[256g:256g+256]^T @ x^T (fp16 partials) and the host
sums them.

Steady state is ACT-bound: the softmax exp (one [128,1024] ACTIVATE per
(k-chunk, head-pair), (N+352)/1.2 ns at 1x) is the per-iteration floor
(~2.3us per k-chunk).  Everything else is arranged to hide under it:
  - scores kept transposed [k, q]; ones-column in the V projection makes the
    AV matmul also produce softmax denominators; probability tiles feed AV
    directly as the moving operand.
  - QK pair matmuls are row-tiled (tile_position via base_partition) so the
    two heads of a pair run concurrently on the PE.
  - ONE psum pool for the whole kernel: scores double-buffer on tag "sT"
    (4 banks, also hosting K/Q projection + normalize/oproj psums via
    rotation), AV accumulators on out0..3 (4 banks, also hosting the V
    projection psums before attention starts).  No pool-context barrier
    between projections and attention.
  - all weights arrive in one batched DMA; qT ships as separate qc0/rest
    tensors so the first q-chunk's projection starts early.
  - per-qc work (normalize, output projection) is spread over the next qc's
    k-loop; Q projection for qc+1 is emitted mid-qc.
  - mask ships host-duplicated ([S, 4096]: each qc block doubled) so one
    [128,1024] DMA + one in-place DVE multiply covers a head-pair.
"""
import ml_dtypes
import numpy as np
from contextlib import ExitStack

import concourse.bass as bass
import concourse.mybir as mybir
import concourse.tile as tile
from concourse import bacc

dt = mybir.dt
AF = mybir.ActivationFunctionType
OP = mybir.AluOpType

B, S, E, H = 2, 2048, 1024, 16
HD = E // H          # 64
HPC = H // 4         # heads per core = 4
N_CORES = 8
ECH = E // 128       # 8 e-chunks
NQ = S // 512        # 4 q chunks
NK = S // 128        # 16 k chunks

# big weights tensor column layout (bf16)
WB_FQ = 0                 # 8 x 256
WB_FK = WB_FQ + 2048      # 8 x 256
WB_FV = WB_FK + 2048      # 8 x 260
WB_WO = WB_FV + 2080      # 2 x 1024
WB_BFV = WB_WO + 2048     # 260 (row 0 only)
WB_ONES = WB_BFV + 260    # 128 (row 0 only)
WB_COLS = WB_ONES + 128   # 6564

_prog_cache = {}


def build_program():
    if "nc" in _prog_cache:
        return _prog_cache["nc"]
    nc = bacc.Bacc("TRN2", target_bir_lowering=False, debug=False,
                   num_devices=N_CORES)

    bf = dt.bfloat16
    WBt = nc.dram_tensor("WB", [128, WB_COLS], bf, kind="ExternalInput").ap()
    WFt = nc.dram_tensor("WF", [128, 4], dt.float32, kind="ExternalInput").ap()
    selt = nc.dram_tensor("sel", [128, 256], dt.float32r, kind="ExternalInput").ap()
    qT = nc.dram_tensor("qT", [E, S], bf, kind="ExternalInput").ap()
    kT = nc.dram_tensor("kT", [E, S], bf, kind="ExternalInput").ap()
    vT = nc.dram_tensor("vT", [E, S], bf, kind="ExternalInput").ap()
    # host-duplicated keep-mask: [S, 4*1024], qc block qc at cols 1024qc,
    # duplicated so one [128,1024] tile covers both heads of a pair.
    maskT2 = nc.dram_tensor("maskT2", [S, 2 * S], bf, kind="ExternalInput").ap()
    out_pT = nc.dram_tensor("out_pT", [E, S], dt.float16, kind="ExternalOutput").ap()
    warm_out = nc.dram_tensor("warm_out", [128, 512], dt.float32, kind="ExternalOutput").ap()

    with tile.TileContext(nc) as tc:
        with ExitStack() as ctx:
            wc = ctx.enter_context(tc.tile_pool(name="wc", bufs=1))
            xin = ctx.enter_context(tc.tile_pool(name="xin", bufs=1))
            qk = ctx.enter_context(tc.tile_pool(name="qk", bufs=1))
            vhp = ctx.enter_context(tc.tile_pool(name="vhp", bufs=1))
            xTp = ctx.enter_context(tc.tile_pool(name="xTp", bufs=1))
            maskp = ctx.enter_context(tc.tile_pool(name="maskp", bufs=4))
            escp = ctx.enter_context(tc.tile_pool(name="escp", bufs=8))
            avnp = ctx.enter_context(tc.tile_pool(name="avnp", bufs=2))
            oev = ctx.enter_context(tc.tile_pool(name="oev", bufs=4))
            psa = ctx.enter_context(tc.tile_pool(name="psa", bufs=1, space="PSUM"))

            # ---- weights: one big bf16 DMA + two small f32 DMAs ----
            WB = wc.tile([128, WB_COLS], bf, tag="WB", name="WB")
            nc.sync.dma_start(WB[:], WBt)
            WF = wc.tile([128, 4], dt.float32, tag="WF")
            nc.sync.dma_start(WF[:], WFt)
            sel_sb = wc.tile([128, 256], dt.float32r, tag="sel")
            nc.sync.dma_start(sel_sb[:], selt)

            def Fq_sb(e):
                return WB[:, bass.ds(WB_FQ + 256 * e, 256)]

            def Fk_sb(e):
                return WB[:, bass.ds(WB_FK + 256 * e, 256)]

            def Fv_sb(e):
                return WB[:, bass.ds(WB_FV + 260 * e, 260)]

            def Wo_sb(c):
                return WB[:, bass.ds(WB_WO + 1024 * c, 1024)]

            bfv_sb = WB[0:1, bass.ds(WB_BFV, 260)]
            onesb_sb = WB[0:1, bass.ds(WB_ONES, 128)]
            bfq_sb = WF[:, 0:2]
            bfk_sb = WF[:, 2:4]

            # ---- input DMAs: qc0 q-slice first, then kT/vT interleaved ----
            qt = [xin.tile([128, S], bf, tag=f"qt{e}", name=f"qt{e}")
                  for e in range(ECH)]
            xt_k = [xin.tile([128, S], bf, tag=f"xk{e}", name=f"xk{e}")
                    for e in range(ECH)]
            xt_v = [xin.tile([128, S], bf, tag=f"xv{e}", name=f"xv{e}")
                    for e in range(ECH)]
            for e in range(ECH):
                nc.sync.dma_start(xt_k[e][:], kT[bass.ts(e, 128), :])
            for e in range(ECH):
                nc.scalar.dma_start(qt[e][:], qT[bass.ts(e, 128), :])
            for e in range(ECH):
                nc.scalar.dma_start(xt_v[e][:], vT[bass.ts(e, 128), :])

            qhT = [qk.tile([128, S], bf, tag=f"qhT{p}", name=f"qhT{p}") for p in range(2)]
            khT = [qk.tile([128, S], bf, tag=f"khT{p}", name=f"khT{p}") for p in range(2)]
            vh_sb = [vhp.tile([128, 4 * 65], bf, tag=f"vh{sc}", name=f"vh{sc}", bufs=1)
                     for sc in range(NK)]
            xT_sb = [xTp.tile([128, S], bf, tag=f"xT{c}", name=f"xT{c}") for c in range(2)]

            # ---- PE warm-up (HAM) on the weights tile ----
            wps = psa.tile([128, 512], dt.float32, tag="sT", name="wps", bufs=2)
            for i in range(12):
                nc.tensor.matmul(wps[:], WB[:, 0:128], WB[:, 0:512],
                                 start=(i == 0), stop=(i == 11))
            wsb = oev.tile([128, 512], dt.float32, tag="wsb", name="wsb")
            nc.vector.tensor_copy(wsb[:], wps[:])
            nc.sync.dma_start(warm_out, wsb[:])

            # ---- K projection (tag sT) ----
            for pair in range(2):
                for nn in range(4):
                    pp = psa.tile([128, 512], dt.float32, tag="sT",
                                  name="pp", bufs=2)
                    for e in range(ECH):
                        nc.tensor.matmul(
                            pp[:],
                            Fk_sb(e)[:, bass.ts(pair, 128)],
                            xt_k[e][:, bass.ts(nn, 512)],
                            start=(e == 0), stop=(e == ECH - 1),
                        )
                    nc.vector.tensor_scalar(
                        khT[pair][:, bass.ts(nn, 512)], pp[:],
                        bfk_sb[:, bass.ds(pair, 1)], None, op0=OP.add,
                    )
            # ---- Q projection, all qc, hoisted to the head ----
            for qcq in range(NQ):
                for pair in range(2):
                    pq = psa.tile([128, 512], dt.float32, tag="sT",
                                  name="pq", bufs=2)
                    for e in range(ECH):
                        nc.tensor.matmul(
                            pq[:],
                            Fq_sb(e)[:, bass.ts(pair, 128)],
                            qt[e][:, bass.ts(qcq, 512)],
                            start=(e == 0), stop=(e == ECH - 1),
                        )
                    nc.vector.tensor_scalar(
                        qhT[pair][:, bass.ts(qcq, 512)], pq[:],
                        bfq_sb[:, bass.ds(pair, 1)], None, op0=OP.add,
                    )

            def emit_vproj(sc):
                pv = psa.tile([128, 260], dt.float32, tag="sT",
                              name="pv", bufs=2)
                for e in range(ECH):
                    nc.tensor.matmul(
                        pv[:], xt_v[e][:, bass.ts(sc, 128)], Fv_sb(e),
                        start=(e == 0), stop=False,
                    )
                nc.tensor.matmul(
                    pv[:], onesb_sb[:, 0:128], bfv_sb,
                    start=False, stop=True,
                )
                nc.vector.tensor_copy(vh_sb[sc][:], pv[:])

            # first two V chunks before the attention loop; the rest are
            # interleaved into qc0's k-loop (sc = kc + 2).
            emit_vproj(0)
            emit_vproj(1)

            # ---- attention ----
            sums128 = avnp.tile([128, 512], dt.float32, tag="sums128",
                                name="sums128", bufs=1)
            nc.vector.memset(sums128[:], 1.0)
            recip128 = avnp.tile([128, 512], dt.float32r, tag="recip128",
                                 name="recip128", bufs=1)
            avs = [avnp.tile([64, 512], dt.float32, tag=f"av{h}",
                             name=f"av{h}", bufs=1) for h in range(HPC)]

            from concourse.dve_ops import (
                RECIP_APPROX_FAST_CONSTS,
                RECIPROCAL_APPROX_FAST,
            )

            def emit_avs(prev_outs, h):
                nc.vector.tensor_copy(avs[h][:], prev_outs[h][0:64, :])
                nc.vector.tensor_copy(sums128[32 * h:32 * h + 1, :],
                                      prev_outs[h][64:65, :])

            def emit_recip(rows=None):
                # reciprocal_approx_fast writing f32r directly (the sel
                # matmul requires an FP32r-rounded producer).
                c = RECIP_APPROX_FAST_CONSTS
                sl = slice(None) if rows is None else rows
                nc.vector._custom_dve(
                    RECIPROCAL_APPROX_FAST,
                    out=recip128[sl, :], in0=sums128[sl, :],
                    s0=c["s0"], s1=c["s1"], imm2=c["imm2"],
                )

            def emit_head_norm(h, pqc):
                pair, lo = h // 2, (h % 2) * 64
                bc = psa.tile([64, 512], dt.float32, tag="sT", name="bc",
                              bufs=2)
                nc.tensor.matmul(bc[:], sel_sb[:, bass.ds(64 * h, 64)],
                                 recip128[:], start=True, stop=True)
                nc.vector.tensor_tensor(
                    xT_sb[pair][lo:lo + 64, bass.ts(pqc, 512)],
                    avs[h][0:64, :], bc[:], op=OP.mult)

            def emit_oproj(eo, pqc, evict="v"):
                po = psa.tile([128, 512], dt.float32, tag="sT",
                              name="po", bufs=2)
                for c in range(2):
                    nc.tensor.matmul(
                        po[:], Wo_sb(c)[:, bass.ts(eo, 128)],
                        xT_sb[c][:, bass.ts(pqc, 512)],
                        start=(c == 0), stop=(c == 1),
                    )
                ot = oev.tile([128, 512], dt.float16, tag="ot", name="ot",
                              bufs=4)
                if evict == "v":
                    nc.vector.tensor_copy(ot[:], po[:])
                else:
                    nc.scalar.copy(ot[:], po[:])
                nc.sync.dma_start(
                    out_pT[bass.ts(eo, 128), bass.ts(pqc, 512)], ot[:])

            prev = None  # outs of previous qc awaiting normalize/oproj
            for qc in range(NQ):
                outs = [psa.tile([65, 512], dt.float32, tag=f"out{h}",
                                 name=f"out{h}") for h in range(HPC)]
                for kc in range(NK):
                    # spread previous-qc normalize/oproj over this qc
                    if prev is not None:
                        if kc == 1:
                            for h in range(HPC):
                                emit_avs(prev, h)
                        elif kc == 2:
                            emit_recip()
                        elif kc in (3, 4):
                            emit_head_norm(2 * (kc - 3), qc - 1)
                            emit_head_norm(2 * (kc - 3) + 1, qc - 1)
                        elif 5 <= kc <= 12:
                            emit_oproj(kc - 5, qc - 1)
                    elif qc == 0 and kc < NK - 2:
                        emit_vproj(kc + 2)

                    mt2 = maskp.tile([128, 1024], bf, tag="mask", name="mask")
                    nc.sync.dma_start(
                        mt2[:],
                        maskT2[bass.ts(kc, 128), bass.ts(qc, 1024)])
                    for pair in range(2):
                        sT = psa.tile([128, 1024], dt.float32, tag="sT",
                                      name="sT", bufs=2)
                        for half in range(2):
                            lo = half * 64
                            nc.tensor.matmul(
                                sT[:, bass.ts(half, 512)],
                                khT[pair][lo:lo + 64, bass.ts(kc, 128)],
                                qhT[pair][lo:lo + 64, bass.ts(qc, 512)],
                                start=True, stop=True,
                            )
                        esc = escp.tile([128, 1024], bf, tag="esc", name="esc")
                        nc.scalar.activation(esc[:], sT[:], AF.Exp)
                        nc.vector.tensor_tensor(
                            esc[:], esc[:], mt2[:], op=OP.mult)
                        for half in range(2):
                            h = 2 * pair + half
                            nc.tensor.matmul(
                                outs[h][:],
                                vh_sb[kc][:, bass.ds(65 * h, 65)],
                                esc[:, bass.ts(half, 512)],
                                start=(kc == 0), stop=(kc == NK - 1),
                            )
                prev = outs

            # tail: last qc; evictions alternate DVE/ACT (ACT is idle here).
            for h in range(HPC):
                emit_avs(prev, h)
            emit_recip()
            for h in range(HPC):
                emit_head_norm(h, NQ - 1)
            for eo in range(ECH):
                emit_oproj(eo, NQ - 1, evict="v" if eo % 2 == 0 else "s")

    nc.compile()
    _prog_cache["nc"] = nc
    return nc


def prep_inputs(q_matrix, k_matrix, v_matrix, mask, Wq, bq, Wk, bk, Wv, bv,
                Whq, bhq, Whk, bhk, Whv, bhv, Wo, bo):
    f32 = np.float32
    bf16 = ml_dtypes.bfloat16
    q_matrix = np.asarray(q_matrix, f32)
    k_matrix = np.asarray(k_matrix, f32)
    v_matrix = np.asarray(v_matrix, f32)
    mask = np.asarray(mask)
    sc = f32(1.0 / np.sqrt(HD))

    Wq, Wk, Wv = np.asarray(Wq, f32), np.asarray(Wk, f32), np.asarray(Wv, f32)
    Whq, Whk, Whv = np.asarray(Whq, f32), np.asarray(Whk, f32), np.asarray(Whv, f32)
    bq, bk, bv = np.asarray(bq, f32), np.asarray(bk, f32), np.asarray(bv, f32)
    bhq, bhk, bhv = np.asarray(bhq, f32), np.asarray(bhk, f32), np.asarray(bhv, f32)
    # Fx[h] = Wx @ Whx[h]: one BLAS call via tensordot -> [E(out), H, HD]
    FqH = (np.tensordot(Wq, Whq, axes=([1], [1])) * sc).astype(f32)
    FkH = np.tensordot(Wk, Whk, axes=([1], [1])).astype(f32)
    FvH = np.tensordot(Wv, Whv, axes=([1], [1])).astype(f32)
    bqH = ((np.einsum("e,hed->hd", bq, Whq) + bhq) * sc).astype(f32)
    bkH = (np.einsum("e,hed->hd", bk, Whk) + bhk).astype(f32)
    bvH = (np.einsum("e,hed->hd", bv, Whv) + bhv).astype(f32)
    WoM = np.asarray(Wo, f32)

    sel = np.zeros((128, 256), f32)
    for h in range(4):
        sel[32 * h, 64 * h:64 * (h + 1)] = 1.0
    in_maps = []
    for core in range(N_CORES):
        b, g = core // 4, core % 4
        hs = [4 * g + j for j in range(4)]
        Fq_c = np.ascontiguousarray(FqH[:, hs, :].reshape(E, 256))
        Fk_c = np.ascontiguousarray(FkH[:, hs, :].reshape(E, 256))
        Fv_c = np.zeros((E, 260), f32)
        bfv_c = np.zeros((260,), f32)
        for j, h in enumerate(hs):
            Fv_c[:, 65 * j:65 * j + 64] = FvH[:, h, :]
            bfv_c[65 * j:65 * j + 64] = bvH[h]
            bfv_c[65 * j + 64] = 1.0
        bfq_c = np.stack([np.concatenate([bqH[hs[2 * p]], bqH[hs[2 * p + 1]]])
                          for p in range(2)], axis=1)                # [128, 2]
        bfk_c = np.stack([np.concatenate([bkH[hs[2 * p]], bkH[hs[2 * p + 1]]])
                          for p in range(2)], axis=1)
        Wo_c = WoM[256 * g:256 * (g + 1), :]                         # [256, 1024]

        # big weights tensor [128, WB_COLS]
        WBm = np.zeros((128, WB_COLS), f32)
        WBm[:, WB_FQ:WB_FQ + 2048] = Fq_c.reshape(ECH, 128, 256).transpose(
            1, 0, 2).reshape(128, 2048)
        WBm[:, WB_FK:WB_FK + 2048] = Fk_c.reshape(ECH, 128, 256).transpose(
            1, 0, 2).reshape(128, 2048)
        WBm[:, WB_FV:WB_FV + 2080] = Fv_c.reshape(ECH, 128, 260).transpose(
            1, 0, 2).reshape(128, 2080)
        WBm[:, WB_WO:WB_WO + 2048] = Wo_c.reshape(2, 128, 1024).transpose(
            1, 0, 2).reshape(128, 2048)
        WBm[0, WB_BFV:WB_BFV + 260] = bfv_c
        WBm[0, WB_ONES:WB_ONES + 128] = 1.0

        WFm = np.concatenate([bfq_c, bfk_c], axis=1)                 # [128, 4]

        # keep-mask, transposed, duplicated per qc block: [S, 4*1024]
        mk = (~mask[b].T).astype(f32)                                # [S(k), S(q)]
        mk2 = np.broadcast_to(
            mk.reshape(S, NQ, 1, 512), (S, NQ, 2, 512)).reshape(S, 2 * S)

        in_maps.append(dict(
            qT=np.ascontiguousarray(q_matrix[b].T).astype(bf16),
            kT=np.ascontiguousarray(k_matrix[b].T).astype(bf16),
            vT=np.ascontiguousarray(v_matrix[b].T).astype(bf16),
            maskT2=np.ascontiguousarray(mk2).astype(bf16),
            WB=WBm.astype(bf16), WF=WFm, sel=sel,
        ))
    return in_maps


def unshard(results, bo):
    bo = np.asarray(bo, np.float32)
    out = np.empty((B, S, E), np.float32)
    for b in range(B):
        acc = results[4 * b]["out_pT"].astype(np.float32)
        for g in range(1, 4):
            acc = acc + results[4 * b + g]["out_pT"].astype(np.float32)
        out[b] = acc.T + bo
    return out


def kernel(**inputs):
    from concourse.bass_utils import run_bass_kernel_spmd
    nc = build_program()
    in_maps = prep_inputs(**inputs)
    res = run_bass_kernel_spmd(nc, in_maps, list(range(N_CORES)))
    return unshard(res.results, inputs["bo"])


# revision 10
# speedup vs baseline: 1.2310x; 1.2310x over previous
"""MultiHeadAttention Trainium2 kernel (8 NeuronCores).

Problem: B=2, S=2048, E=1024, H=16, HD=64.
  qg = q @ Wq + bq ; qh[h] = qg @ Whq[h] + bhq[h]   (same for k, v)
  scores = qh @ kh^T / sqrt(HD), masked (-inf where mask), softmax
  out = concat_h(softmax @ vh) @ Wo + bo

Sharding: core c = 4*b + g handles batch b, heads 4g..4g+3 (data parallel on
B, tensor parallel on H). The global+per-head projections are folded on the
host into per-head fused weights Fq[h] = Wq @ Whq[h] (etc.), so each core
runs one [E, HD] projection per head. The output projection is row-sharded:
each core computes Wo[256g:256g+256]^T @ x^T (fp16 partials) and the host
sums them.

Steady state is ACT-bound: the softmax exp (one [128,1024] ACTIVATE per
(k-chunk, head-pair), (N+352)/1.2 ns at 1x) is the per-iteration floor
(~2.3us per k-chunk).  Everything else is arranged to hide under it:
  - scores kept transposed [k, q]; ones-column in the V projection makes the
    AV matmul also produce softmax denominators; probability tiles feed AV
    directly as the moving operand.
  - QK pair matmuls are row-tiled (tile_position via base_partition) so the
    two heads of a pair run concurrently on the PE.
  - ONE psum pool for the whole kernel: scores double-buffer on tag "sT"
    (4 banks, also hosting K/Q projection + normalize/oproj psums via
    rotation), AV accumulators on out0..3 (4 banks, also hosting the V
    projection psums before attention starts).  No pool-context barrier
    between projections and attention.
  - all weights arrive in one batched DMA; qT ships as separate qc0/rest
    tensors so the first q-chunk's projection starts early.
  - per-qc work (normalize, output projection) is spread over the next qc's
    k-loop; Q projection for qc+1 is emitted mid-qc.
  - mask ships host-duplicated ([S, 4096]: each qc block doubled) so one
    [128,1024] DMA + one in-place DVE multiply covers a head-pair.
"""
import ml_dtypes
import numpy as np
from contextlib import ExitStack

import concourse.bass as bass
import concourse.mybir as mybir
import concourse.tile as tile
from concourse import bacc

dt = mybir.dt
AF = mybir.ActivationFunctionType
OP = mybir.AluOpType

B, S, E, H = 2, 2048, 1024, 16
HD = E // H          # 64
HPC = H // 4         # heads per core = 4
N_CORES = 8
ECH = E // 128       # 8 e-chunks
NQ = S // 512        # 4 q chunks
NK = S // 128        # 16 k chunks

# big weights tensor column layout (bf16)
WB_FQ = 0                 # 8 x 256
WB_FK = WB_FQ + 2048      # 8 x 256
WB_FV = WB_FK + 2048      # 8 x 260
WB_WO = WB_FV + 2080      # 2 x 1024
WB_BFV = WB_WO + 2048     # 260 (row 0 only)
WB_ONES = WB_BFV + 260    # 128 (row 0 only)
WB_COLS = WB_ONES + 128   # 6564

_prog_cache = {}


def build_program():
    if "nc" in _prog_cache:
        return _prog_cache["nc"]
    nc = bacc.Bacc("TRN2", target_bir_lowering=False, debug=False,
                   num_devices=N_CORES)

    bf = dt.bfloat16
    WBt = nc.dram_tensor("WB", [128, WB_COLS], bf, kind="ExternalInput").ap()
    WFt = nc.dram_tensor("WF", [128, 4], dt.float32, kind="ExternalInput").ap()
    selt = nc.dram_tensor("sel", [128, 256], dt.float32r, kind="ExternalInput").ap()
    qT = nc.dram_tensor("qT", [E, S], bf, kind="ExternalInput").ap()
    kT = nc.dram_tensor("kT", [E, S], bf, kind="ExternalInput").ap()
    vT = nc.dram_tensor("vT", [E, S], bf, kind="ExternalInput").ap()
    # host-duplicated keep-mask: [S, 4*1024], qc block qc at cols 1024qc,
    # duplicated so one [128,1024] tile covers both heads of a pair.
    maskT2 = nc.dram_tensor("maskT2", [S, 2 * S], bf, kind="ExternalInput").ap()
    out_pT = nc.dram_tensor("out_pT", [E, S], dt.float16, kind="ExternalOutput").ap()
    warm_out = nc.dram_tensor("warm_out", [128, 512], dt.float32, kind="ExternalOutput").ap()

    with tile.TileContext(nc) as tc:
        with ExitStack() as ctx:
            wc = ctx.enter_context(tc.tile_pool(name="wc", bufs=1))
            xin = ctx.enter_context(tc.tile_pool(name="xin", bufs=1))
            qk = ctx.enter_context(tc.tile_pool(name="qk", bufs=1))
            vhp = ctx.enter_context(tc.tile_pool(name="vhp", bufs=1))
            xTp = ctx.enter_context(tc.tile_pool(name="xTp", bufs=1))
            maskp = ctx.enter_context(tc.tile_pool(name="maskp", bufs=4))
            escp = ctx.enter_context(tc.tile_pool(name="escp", bufs=8))
            avnp = ctx.enter_context(tc.tile_pool(name="avnp", bufs=2))
            oev = ctx.enter_context(tc.tile_pool(name="oev", bufs=4))
            psa = ctx.enter_context(tc.tile_pool(name="psa", bufs=1, space="PSUM"))

            # ---- weights: one big bf16 DMA + two small f32 DMAs ----
            WB = wc.tile([128, WB_COLS], bf, tag="WB", name="WB")
            nc.sync.dma_start(WB[:], WBt)
            WF = wc.tile([128, 4], dt.float32, tag="WF")
            nc.sync.dma_start(WF[:], WFt)
            sel_sb = wc.tile([128, 256], dt.float32r, tag="sel")
            nc.sync.dma_start(sel_sb[:], selt)

            def Fq_sb(e):
                return WB[:, bass.ds(WB_FQ + 256 * e, 256)]

            def Fk_sb(e):
                return WB[:, bass.ds(WB_FK + 256 * e, 256)]

            def Fv_sb(e):
                return WB[:, bass.ds(WB_FV + 260 * e, 260)]

            def Wo_sb(c):
                return WB[:, bass.ds(WB_WO + 1024 * c, 1024)]

            bfv_sb = WB[0:1, bass.ds(WB_BFV, 260)]
            onesb_sb = WB[0:1, bass.ds(WB_ONES, 128)]
            bfq_sb = WF[:, 0:2]
            bfk_sb = WF[:, 2:4]

            # ---- input DMAs: qc0 q-slice first, then kT/vT interleaved ----
            qt = [xin.tile([128, S], bf, tag=f"qt{e}", name=f"qt{e}")
                  for e in range(ECH)]
            xt_k = [xin.tile([128, S], bf, tag=f"xk{e}", name=f"xk{e}")
                    for e in range(ECH)]
            xt_v = [xin.tile([128, S], bf, tag=f"xv{e}", name=f"xv{e}")
                    for e in range(ECH)]
            for e in range(ECH):
                nc.sync.dma_start(xt_k[e][:], kT[bass.ts(e, 128), :])
                nc.scalar.dma_start(xt_v[e][:], vT[bass.ts(e, 128), :])
            for e in range(ECH):
                nc.sync.dma_start(qt[e][:], qT[bass.ts(e, 128), :])

            qhT = [qk.tile([128, S], bf, tag=f"qhT{p}", name=f"qhT{p}") for p in range(2)]
            khT = [qk.tile([128, S], bf, tag=f"khT{p}", name=f"khT{p}") for p in range(2)]
            vh_sb = [vhp.tile([128, 4 * 65], bf, tag=f"vh{sc}", name=f"vh{sc}", bufs=1)
                     for sc in range(NK)]
            xT_sb = [xTp.tile([128, S], bf, tag=f"xT{c}", name=f"xT{c}") for c in range(2)]

            # ---- PE warm-up (HAM) on the weights tile ----
            wps = psa.tile([128, 512], dt.float32, tag="sT", name="wps", bufs=2)
            for i in range(12):
                nc.tensor.matmul(wps[:], WB[:, 0:128], WB[:, 0:512],
                                 start=(i == 0), stop=(i == 11))
            wsb = oev.tile([128, 512], dt.float32, tag="wsb", name="wsb")
            nc.vector.tensor_copy(wsb[:], wps[:])
            nc.sync.dma_start(warm_out, wsb[:])

            # ---- K projection (tag sT) ----
            for pair in range(2):
                for nn in range(4):
                    pp = psa.tile([128, 512], dt.float32, tag="sT",
                                  name="pp", bufs=2)
                    for e in range(ECH):
                        nc.tensor.matmul(
                            pp[:],
                            Fk_sb(e)[:, bass.ts(pair, 128)],
                            xt_k[e][:, bass.ts(nn, 512)],
                            start=(e == 0), stop=(e == ECH - 1),
                        )
                    nc.vector.tensor_scalar(
                        khT[pair][:, bass.ts(nn, 512)], pp[:],
                        bfk_sb[:, bass.ds(pair, 1)], None, op0=OP.add,
                    )
            # ---- Q projection, all qc, hoisted to the head ----
            for qcq in range(NQ):
                for pair in range(2):
                    pq = psa.tile([128, 512], dt.float32, tag="sT",
                                  name="pq", bufs=2)
                    for e in range(ECH):
                        nc.tensor.matmul(
                            pq[:],
                            Fq_sb(e)[:, bass.ts(pair, 128)],
                            qt[e][:, bass.ts(qcq, 512)],
                            start=(e == 0), stop=(e == ECH - 1),
                        )
                    nc.vector.tensor_scalar(
                        qhT[pair][:, bass.ts(qcq, 512)], pq[:],
                        bfq_sb[:, bass.ds(pair, 1)], None, op0=OP.add,
                    )

            def emit_vproj(sc):
                pv = psa.tile([128, 260], dt.float32, tag="sT",
                              name="pv", bufs=2)
                for e in range(ECH):
                    nc.tensor.matmul(
                        pv[:], xt_v[e][:, bass.ts(sc, 128)], Fv_sb(e),
                        start=(e == 0), stop=False,
                    )
                nc.tensor.matmul(
                    pv[:], onesb_sb[:, 0:128], bfv_sb,
                    start=False, stop=True,
                )
                with tc.high_priority():
                    nc.vector.tensor_copy(vh_sb[sc][:], pv[:])

            # first two V chunks before the attention loop; the rest are
            # interleaved into qc0's k-loop (sc = kc + 2).
            emit_vproj(0)
            emit_vproj(1)

            # ---- attention ----
            sums128 = avnp.tile([128, 512], dt.float32, tag="sums128",
                                name="sums128", bufs=1)
            nc.vector.memset(sums128[:], 1.0)
            recip128 = avnp.tile([128, 512], dt.float32r, tag="recip128",
                                 name="recip128", bufs=1)
            avs = [avnp.tile([64, 512], dt.float32, tag=f"av{h}",
                             name=f"av{h}", bufs=1) for h in range(HPC)]

            from concourse.dve_ops import (
                RECIP_APPROX_FAST_CONSTS,
                RECIPROCAL_APPROX_FAST,
            )

            def emit_avs(prev_outs, h):
                with tc.high_priority():
                    nc.vector.tensor_copy(avs[h][:], prev_outs[h][0:64, :])
                    nc.vector.tensor_copy(sums128[32 * h:32 * h + 1, :],
                                          prev_outs[h][64:65, :])

            def emit_recip(rows=None):
                # reciprocal_approx_fast writing f32r directly (the sel
                # matmul requires an FP32r-rounded producer).
                c = RECIP_APPROX_FAST_CONSTS
                sl = slice(None) if rows is None else rows
                with tc.high_priority():
                    nc.vector._custom_dve(
                        RECIPROCAL_APPROX_FAST,
                        out=recip128[sl, :], in0=sums128[sl, :],
                        s0=c["s0"], s1=c["s1"], imm2=c["imm2"],
                    )

            def emit_head_norm(h, pqc):
                pair, lo = h // 2, (h % 2) * 64
                bc = psa.tile([64, 512], dt.float32, tag="sT", name="bc",
                              bufs=2)
                nc.tensor.matmul(bc[:], sel_sb[:, bass.ds(64 * h, 64)],
                                 recip128[:], start=True, stop=True)
                with tc.high_priority():
                    nc.vector.tensor_tensor(
                        xT_sb[pair][lo:lo + 64, bass.ts(pqc, 512)],
                        avs[h][0:64, :], bc[:], op=OP.mult)

            def emit_oproj(eo, pqc, evict="v"):
                po = psa.tile([128, 512], dt.float32, tag="sT",
                              name="po", bufs=2)
                for c in range(2):
                    nc.tensor.matmul(
                        po[:], Wo_sb(c)[:, bass.ts(eo, 128)],
                        xT_sb[c][:, bass.ts(pqc, 512)],
                        start=(c == 0), stop=(c == 1),
                    )
                ot = oev.tile([128, 512], dt.float16, tag="ot", name="ot",
                              bufs=4)
                with tc.high_priority():
                    if evict == "v":
                        nc.vector.tensor_copy(ot[:], po[:])
                    else:
                        nc.scalar.copy(ot[:], po[:])
                nc.sync.dma_start(
                    out_pT[bass.ts(eo, 128), bass.ts(pqc, 512)], ot[:])

            prev = None  # outs of previous qc awaiting normalize/oproj
            for qc in range(NQ):
                outs = [psa.tile([65, 512], dt.float32, tag=f"out{h}",
                                 name=f"out{h}") for h in range(HPC)]
                for kc in range(NK):
                    mt2 = maskp.tile([128, 1024], bf, tag="mask", name="mask")
                    nc.sync.dma_start(
                        mt2[:],
                        maskT2[bass.ts(kc, 128), bass.ts(qc, 1024)])
                    for pair in range(2):
                        sT = psa.tile([128, 1024], dt.float32, tag="sT",
                                      name="sT", bufs=2)
                        for half in range(2):
                            lo = half * 64
                            nc.tensor.matmul(
                                sT[:, bass.ts(half, 512)],
                                khT[pair][lo:lo + 64, bass.ts(kc, 128)],
                                qhT[pair][lo:lo + 64, bass.ts(qc, 512)],
                                start=True, stop=True,
                            )
                        esc = escp.tile([128, 1024], bf, tag="esc", name="esc")
                        nc.scalar.activation(esc[:], sT[:], AF.Exp)
                        nc.vector.tensor_tensor(
                            esc[:], esc[:], mt2[:], op=OP.mult)
                        for half in range(2):
                            h = 2 * pair + half
                            nc.tensor.matmul(
                                outs[h][:],
                                vh_sb[kc][:, bass.ds(65 * h, 65)],
                                esc[:, bass.ts(half, 512)],
                                start=(kc == 0), stop=(kc == NK - 1),
                            )
                    # interleaved extras, emitted at the END of the kc body
                    # so their psum-slot wait lands a full period later.
                    if prev is not None:
                        if kc == 0:
                            for h in range(HPC):
                                emit_avs(prev, h)
                        elif kc == 1:
                            emit_recip()
                        elif kc in (2, 3):
                            emit_head_norm(2 * (kc - 2), qc - 1)
                            emit_head_norm(2 * (kc - 2) + 1, qc - 1)
                        elif 4 <= kc <= 11:
                            emit_oproj(kc - 4, qc - 1)
                    elif qc == 0 and kc < NK - 2:
                        emit_vproj(kc + 2)
                prev = outs

            # tail: last qc; evictions alternate DVE/ACT (ACT is idle here).
            for h in range(HPC):
                emit_avs(prev, h)
            emit_recip()
            for h in range(HPC):
                emit_head_norm(h, NQ - 1)
            for eo in range(ECH):
                emit_oproj(eo, NQ - 1, evict="v" if eo % 2 == 0 else "s")

    nc.compile()
    _prog_cache["nc"] = nc
    return nc


def prep_inputs(q_matrix, k_matrix, v_matrix, mask, Wq, bq, Wk, bk, Wv, bv,
                Whq, bhq, Whk, bhk, Whv, bhv, Wo, bo):
    f32 = np.float32
    bf16 = ml_dtypes.bfloat16
    q_matrix = np.asarray(q_matrix, f32)
    k_matrix = np.asarray(k_matrix, f32)
    v_matrix = np.asarray(v_matrix, f32)
    mask = np.asarray(mask)
    sc = f32(1.0 / np.sqrt(HD))

    Wq, Wk, Wv = np.asarray(Wq, f32), np.asarray(Wk, f32), np.asarray(Wv, f32)
    Whq, Whk, Whv = np.asarray(Whq, f32), np.asarray(Whk, f32), np.asarray(Whv, f32)
    bq, bk, bv = np.asarray(bq, f32), np.asarray(bk, f32), np.asarray(bv, f32)
    bhq, bhk, bhv = np.asarray(bhq, f32), np.asarray(bhk, f32), np.asarray(bhv, f32)
    # Fx[h] = Wx @ Whx[h]: one BLAS call via tensordot -> [E(out), H, HD]
    FqH = (np.tensordot(Wq, Whq, axes=([1], [1])) * sc).astype(f32)
    FkH = np.tensordot(Wk, Whk, axes=([1], [1])).astype(f32)
    FvH = np.tensordot(Wv, Whv, axes=([1], [1])).astype(f32)
    bqH = ((np.einsum("e,hed->hd", bq, Whq) + bhq) * sc).astype(f32)
    bkH = (np.einsum("e,hed->hd", bk, Whk) + bhk).astype(f32)
    bvH = (np.einsum("e,hed->hd", bv, Whv) + bhv).astype(f32)
    WoM = np.asarray(Wo, f32)

    sel = np.zeros((128, 256), f32)
    for h in range(4):
        sel[32 * h, 64 * h:64 * (h + 1)] = 1.0
    in_maps = []
    for core in range(N_CORES):
        b, g = core // 4, core % 4
        hs = [4 * g + j for j in range(4)]
        Fq_c = np.ascontiguousarray(FqH[:, hs, :].reshape(E, 256))
        Fk_c = np.ascontiguousarray(FkH[:, hs, :].reshape(E, 256))
        Fv_c = np.zeros((E, 260), f32)
        bfv_c = np.zeros((260,), f32)
        for j, h in enumerate(hs):
            Fv_c[:, 65 * j:65 * j + 64] = FvH[:, h, :]
            bfv_c[65 * j:65 * j + 64] = bvH[h]
            bfv_c[65 * j + 64] = 1.0
        bfq_c = np.stack([np.concatenate([bqH[hs[2 * p]], bqH[hs[2 * p + 1]]])
                          for p in range(2)], axis=1)                # [128, 2]
        bfk_c = np.stack([np.concatenate([bkH[hs[2 * p]], bkH[hs[2 * p + 1]]])
                          for p in range(2)], axis=1)
        Wo_c = WoM[256 * g:256 * (g + 1), :]                         # [256, 1024]

        # big weights tensor [128, WB_COLS]
        WBm = np.zeros((128, WB_COLS), f32)
        WBm[:, WB_FQ:WB_FQ + 2048] = Fq_c.reshape(ECH, 128, 256).transpose(
            1, 0, 2).reshape(128, 2048)
        WBm[:, WB_FK:WB_FK + 2048] = Fk_c.reshape(ECH, 128, 256).transpose(
            1, 0, 2).reshape(128, 2048)
        WBm[:, WB_FV:WB_FV + 2080] = Fv_c.reshape(ECH, 128, 260).transpose(
            1, 0, 2).reshape(128, 2080)
        WBm[:, WB_WO:WB_WO + 2048] = Wo_c.reshape(2, 128, 1024).transpose(
            1, 0, 2).reshape(128, 2048)
        WBm[0, WB_BFV:WB_BFV + 260] = bfv_c
        WBm[0, WB_ONES:WB_ONES + 128] = 1.0

        WFm = np.concatenate([bfq_c, bfk_c], axis=1)                 # [128, 4]

        # keep-mask, transposed, duplicated per qc block: [S, 4*1024]
        mk = (~mask[b].T).astype(f32)                                # [S(k), S(q)]
        mk2 = np.broadcast_to(
            mk.reshape(S, NQ, 1, 512), (S, NQ, 2, 512)).reshape(S, 2 * S)

        in_maps.append(dict(
            qT=np.ascontiguousarray(q_matrix[b].T).astype(bf16),
            kT=np.ascontiguousarray(k_matrix[b].T).astype(bf16),
            vT=np.ascontiguousarray(v_matrix[b].T).astype(bf16),
            maskT2=np.ascontiguousarray(mk2).astype(bf16),
            WB=WBm.astype(bf16), WF=WFm, sel=sel,
        ))
    return in_maps


def unshard(results, bo):
    bo = np.asarray(bo, np.float32)
    out = np.empty((B, S, E), np.float32)
    for b in range(B):
        acc = results[4 * b]["out_pT"].astype(np.float32)
        for g in range(1, 4):
            acc = acc + results[4 * b + g]["out_pT"].astype(np.float32)
        out[b] = acc.T + bo
    return out


def kernel(**inputs):
    from concourse.bass_utils import run_bass_kernel_spmd
    nc = build_program()
    in_maps = prep_inputs(**inputs)
    res = run_bass_kernel_spmd(nc, in_maps, list(range(N_CORES)))
    return unshard(res.results, inputs["bo"])
